# revision 1
# baseline (speedup 1.0000x reference)
"""Causal multi-head attention on 8 Trainium2 NeuronCores.

Problem: B=4, T=2048, D=2048, H=16 heads, HD=128.
  q = x@Wq.T, k = x@Wk.T, v = x@Wv.T  (per-head causal softmax(q k^T/sqrt(hd)) v)
  out = ctx@Wo.T + b_out

Sharding: batch(4) x head-group(2) grid over 8 cores. Core c handles batch
b=c//2 and heads [8g, 8g+8) with g=c%2. Wq/Wk/Wv split column-wise (head
slices), Wo row-wise; each core emits a partial [T, D] output and the host
sums pairs (row-parallel all-reduce done on host) and adds b_out.

Per-core kernel (all matmuls in float32r: full PE rate at free-dim>=256):
  Phase 1: QT/KT projections in [hd, T] layout and V in [T, hd] layout,
           staged to DRAM. x.T is provided by the host so the contraction
           dim D lands on partitions.
  Phase 2: per head: scores computed transposed (sT[k,q] = K_chunk @ QT),
           p = exp(sT - 10) on ScalarE (scores are O(1); fixed -10 offset
           makes overflow impossible up to s=98 without a max pass),
           causal mask via affine_select fill-0 after exp, softmax sums via
           ones-matmul into PSUM, PV matmul accumulated over k-tiles, then
           deferred normalization: ctx *= partition_broadcast(1/l).
           ctx (f32r) staged to DRAM.
  Phase 3: out[t, :] = sum_h ctxT_h[:, t].T @ WoT_h  accumulated in PSUM.

The 1/sqrt(HD) score scale is folded into Wq on the host.
"""

import math
import numpy as np
from contextlib import ExitStack

import concourse.bacc as bacc
import concourse.mybir as mybir
import concourse.tile as tile
from concourse.bass_utils import run_bass_kernel_spmd

B, T, D = 4, 2048, 2048
H, HD = 16, 128
P = 128
N_CORES = 8
HEADS_PER_CORE = H // 2          # 8 heads per core (head-group split)
DL = HEADS_PER_CORE * HD         # 1024 local projection dims per core
KC = D // P                      # 16 contraction chunks
TT = T // P                      # 16 token tiles of 128
TC4 = T // 512                   # 4 token chunks of 512
EXP_BIAS = -10.0                 # exp(s + EXP_BIAS); cancels in normalization

F32 = mybir.dt.float32
F32R = mybir.dt.float32r

_CACHE = {}


def _build(repeat=1):
    nc = bacc.Bacc(None, target_bir_lowering=False)

    xT = nc.dram_tensor("xT", [D, T], F32R, kind="ExternalInput")
    wqT = nc.dram_tensor("wqT", [D, DL], F32R, kind="ExternalInput")
    wkT = nc.dram_tensor("wkT", [D, DL], F32R, kind="ExternalInput")
    wvT = nc.dram_tensor("wvT", [D, DL], F32R, kind="ExternalInput")
    woT = nc.dram_tensor("woT", [DL, D], F32R, kind="ExternalInput")
    out = nc.dram_tensor("out", [T, D], F32, kind="ExternalOutput")

    with tile.TileContext(nc) as tc:
        with ExitStack() as octx:
            dram = octx.enter_context(tc.tile_pool(name="dram", bufs=1, space="DRAM"))
            qT_st = dram.tile([HEADS_PER_CORE, P, T], F32R)
            kT_st = dram.tile([HEADS_PER_CORE, P, T], F32R)
            v_st = dram.tile([T, DL], F32R)
            ctx_st = dram.tile([HEADS_PER_CORE, P, T], F32R)

            for _rep in range(repeat):
                # ---------------- Phase 1: projections ----------------
                with ExitStack() as p1:
                    xp = p1.enter_context(tc.tile_pool(name="xp", bufs=1))
                    wp = p1.enter_context(tc.tile_pool(name="wp", bufs=2))
                    cp = p1.enter_context(tc.tile_pool(name="cp", bufs=3))
                    ps1 = p1.enter_context(tc.tile_pool(name="ps1", bufs=4, space="PSUM"))

                    x_t = []
                    for kc in range(KC):
                        xt = xp.tile([P, T], F32R, tag=f"x{kc}", name=f"x{kc}")
                        nc.sync.dma_start(xt[:], xT[kc * P:(kc + 1) * P, :])
                        x_t.append(xt)

                    # V projection -> [T, hd] natural layout, 256-wide out chunks
                    for m2 in range(DL // 256):
                        wv = wp.tile([P, KC, 256], F32R, tag="wv")
                        nc.sync.dma_start(
                            wv[:],
                            wvT.rearrange("(kc p) f -> p kc f", p=P)[
                                :, :, m2 * 256:(m2 + 1) * 256
                            ],
                        )
                        for tt in range(TT):
                            ps = ps1.tile([P, 256], F32, tag="ps_v")
                            for kc in range(KC):
                                nc.tensor.matmul(
                                    ps[:],
                                    x_t[kc][:, tt * P:(tt + 1) * P],
                                    wv[:, kc, :],
                                    start=(kc == 0),
                                    stop=(kc == KC - 1),
                                )
                            st = cp.tile([P, 256], F32R, tag="stv")
                            nc.vector.tensor_copy(st[:], ps[:])
                            nc.sync.dma_start(
                                v_st[tt * P:(tt + 1) * P, m2 * 256:(m2 + 1) * 256], st[:]
                            )

                    # Q/K projections -> [hd, T] per head
                    for m in range(HEADS_PER_CORE):
                        for wsrc, dst in ((wqT, qT_st), (wkT, kT_st)):
                            wm = wp.tile([P, KC, P], F32R, tag="wqk")
                            nc.sync.dma_start(
                                wm[:],
                                wsrc.rearrange("(kc p) f -> p kc f", p=P)[
                                    :, :, m * P:(m + 1) * P
                                ],
                            )
                            for t4 in range(TC4):
                                ps = ps1.tile([P, 512], F32, tag="ps_qk")
                                for kc in range(KC):
                                    nc.tensor.matmul(
                                        ps[:],
                                        wm[:, kc, :],
                                        x_t[kc][:, t4 * 512:(t4 + 1) * 512],
                                        start=(kc == 0),
                                        stop=(kc == KC - 1),
                                    )
                                st = cp.tile([P, 512], F32R, tag="stqk")
                                nc.vector.tensor_copy(st[:], ps[:])
                                nc.sync.dma_start(
                                    dst[m][:, t4 * 512:(t4 + 1) * 512], st[:]
                                )

                # ---------------- Phase 2: attention per head ----------------
                with ExitStack() as p2:
                    qkv = p2.enter_context(tc.tile_pool(name="qkv", bufs=2))
                    pTp = p2.enter_context(tc.tile_pool(name="pTp", bufs=2))
                    msc = p2.enter_context(tc.tile_pool(name="msc", bufs=3))
                    ps_s = p2.enter_context(tc.tile_pool(name="ps_s", bufs=3, space="PSUM"))
                    ps_l = p2.enter_context(tc.tile_pool(name="ps_l", bufs=2, space="PSUM"))
                    ps_c = p2.enter_context(tc.tile_pool(name="ps_c", bufs=2, space="PSUM"))

                    ones_f = msc.tile([P, 1], F32, tag="ones_f")
                    nc.vector.memset(ones_f[:], 1.0)
                    ones = msc.tile([P, 1], F32R, tag="ones")
                    nc.vector.tensor_copy(ones[:], ones_f[:])
                    ebias = msc.tile([P, 1], F32, tag="ebias")
                    nc.vector.memset(ebias[:], EXP_BIAS)

                    for h in range(HEADS_PER_CORE):
                        qh = qkv.tile([P, T], F32R, tag="qh")
                        kh = qkv.tile([P, T], F32R, tag="kh")
                        vh = qkv.tile([P, TT, P], F32R, tag="vh")
                        nc.sync.dma_start(qh[:], qT_st[h])
                        nc.sync.dma_start(kh[:], kT_st[h])
                        nc.sync.dma_start(
                            vh[:],
                            v_st.rearrange("(kt p) m -> p kt m", p=P)[
                                :, :, h * P:(h + 1) * P
                            ],
                        )

                        for qc in range(TC4):
                            nkt = 4 * qc + 4
                            pT_t = [pTp.tile([P, 512], F32R, tag=f"pT{ki}", name=f"pT{ki}")
                                    for ki in range(nkt)]
                            l_ps = ps_l.tile([1, 512], F32, tag="l")
                            c_ps = ps_c.tile([P, 512], F32, tag="c")
                            for ki in range(nkt):
                                s_ps = ps_s.tile([P, 512], F32, tag="s")
                                nc.tensor.matmul(
                                    s_ps[:],
                                    kh[:, ki * P:(ki + 1) * P],
                                    qh[:, qc * 512:(qc + 1) * 512],
                                    start=True,
                                    stop=True,
                                )
                                nc.scalar.activation(
                                    pT_t[ki][:], s_ps[:],
                                    mybir.ActivationFunctionType.Exp,
                                    bias=ebias[:], scale=1.0,
                                )
                                j = ki - 4 * qc
                                if j >= 0:
                                    # keep iff q_rel - k_rel - 128*j >= 0
                                    nc.gpsimd.affine_select(
                                        out=pT_t[ki][:], in_=pT_t[ki][:],
                                        compare_op=mybir.AluOpType.is_ge,
                                        fill=0.0, base=-P * j,
                                        channel_multiplier=-1,
                                        pattern=[[1, 512]],
                                    )
                                nc.tensor.matmul(
                                    l_ps[:], ones[:], pT_t[ki][:],
                                    start=(ki == 0), stop=(ki == nkt - 1),
                                )
                                nc.tensor.matmul(
                                    c_ps[:], vh[:, ki, :], pT_t[ki][:],
                                    start=(ki == 0), stop=(ki == nkt - 1),
                                )
                            rl = msc.tile([1, 512], F32, tag="rl")
                            nc.vector.reciprocal(rl[:], l_ps[:])
                            rb = msc.tile([P, 512], F32, tag="rb")
                            nc.gpsimd.partition_broadcast(rb[:], rl[:])
                            cst = msc.tile([P, 512], F32R, tag="cst")
                            nc.vector.tensor_mul(cst[:], c_ps[:], rb[:])
                            nc.sync.dma_start(
                                ctx_st[h][:, qc * 512:(qc + 1) * 512], cst[:]
                            )

                # ---------------- Phase 3: output projection ----------------
                with ExitStack() as p3:
                    wop = p3.enter_context(tc.tile_pool(name="wop", bufs=1))
                    ctxp = p3.enter_context(tc.tile_pool(name="ctxp", bufs=1))
                    ocp = p3.enter_context(tc.tile_pool(name="ocp", bufs=3))
                    ps3 = p3.enter_context(tc.tile_pool(name="ps3", bufs=4, space="PSUM"))

                    wo_t, ctx_t = [], []
                    for h in range(HEADS_PER_CORE):
                        wt = wop.tile([P, D], F32R, tag=f"wo{h}", name=f"wo{h}")
                        ct = ctxp.tile([P, T], F32R, tag=f"cx{h}", name=f"cx{h}")
                        nc.sync.dma_start(wt[:], woT[h * P:(h + 1) * P, :])
                        nc.sync.dma_start(ct[:], ctx_st[h])
                        wo_t.append(wt)
                        ctx_t.append(ct)

                    for tt in range(TT):
                        for oc in range(D // 512):
                            ps = ps3.tile([P, 512], F32, tag="ps_o")
                            for h in range(HEADS_PER_CORE):
                                nc.tensor.matmul(
                                    ps[:],
                                    ctx_t[h][:, tt * P:(tt + 1) * P],
                                    wo_t[h][:, oc * 512:(oc + 1) * 512],
                                    start=(h == 0),
                                    stop=(h == HEADS_PER_CORE - 1),
                                )
                            ot = ocp.tile([P, 512], F32, tag="ot")
                            nc.vector.tensor_copy(ot[:], ps[:])
                            nc.sync.dma_start(
                                out[tt * P:(tt + 1) * P, oc * 512:(oc + 1) * 512], ot[:]
                            )

    nc.compile()
    return nc


def _get_nc(repeat=1):
    if repeat not in _CACHE:
        _CACHE[repeat] = _build(repeat)
    return _CACHE[repeat]


def run(inputs, trace=False, repeat=1):
    x = np.asarray(inputs["x"], dtype=np.float32)
    Wq = np.asarray(inputs["Wq"], dtype=np.float32)
    Wk = np.asarray(inputs["Wk"], dtype=np.float32)
    Wv = np.asarray(inputs["Wv"], dtype=np.float32)
    Wo = np.asarray(inputs["Wo"], dtype=np.float32)
    b_out = np.asarray(inputs["b_out"], dtype=np.float32)

    scale = 1.0 / math.sqrt(HD)
    in_maps = []
    for c in range(N_CORES):
        b, g = divmod(c, 2)
        hs = slice(g * DL, (g + 1) * DL)
        in_maps.append({
            "xT": np.ascontiguousarray(x[b].T),
            "wqT": np.ascontiguousarray((Wq[hs, :] * scale).T),
            "wkT": np.ascontiguousarray(Wk[hs, :].T),
            "wvT": np.ascontiguousarray(Wv[hs, :].T),
            "woT": np.ascontiguousarray(Wo[:, hs].T),
        })

    nc = _get_nc(repeat)
    res = run_bass_kernel_spmd(nc, in_maps, core_ids=list(range(N_CORES)),
                               trace=trace)
    outp = np.empty((B, T, D), dtype=np.float32)
    for b in range(B):
        outp[b] = res.results[2 * b]["out"] + res.results[2 * b + 1]["out"]
    outp += b_out[None, None, :]
    return outp, res


def kernel(**inputs) -> np.ndarray:
    outp, _ = run(inputs, trace=False)
    return outp



# revision 3
# speedup vs baseline: 1.9145x; 1.9145x over previous
"""Causal multi-head attention on 8 Trainium2 NeuronCores.

Problem: B=4, T=2048, D=2048, H=16 heads, HD=128.
  q = x@Wq.T, k = x@Wk.T, v = x@Wv.T  (per-head causal softmax(q k^T/sqrt(hd)) v)
  out = ctx@Wo.T + b_out

Sharding: batch(4) x head-group(2) grid over 8 cores. Core c handles batch
b=c//2 and heads [8g, 8g+8) with g=c%2. Wq/Wk/Wv split column-wise (head
slices), Wo row-wise; each core emits a partial [T, D] output (bf16) and the
host sums pairs in f32 and adds b_out.

All matmul operands are bf16 (full PE rate, half the DMA/SBUF of f32); PSUM
accumulation is f32. Everything stages through SBUF — no DRAM round trips:

  P1 (projections, per head h): qT_h/kT_h in [hd, T] layout, V in [T, hd]
     pair tiles. x.T is host-provided so contraction D lands on partitions.
  P2 (attention, head h emitted with P1 of head h+2 so the tensor engine
     stays dense while ACT runs exp): scores transposed sT[k,q] = K_chunk @
     QT, p = exp(sT - 10) on ACT (scores are O(1); the fixed offset makes a
     max pass unnecessary), causal mask via affine_select fill-0 after exp,
     softmax sums via ones-matmul into PSUM, PV accumulated over k-tiles,
     deferred normalization ctx *= partition_broadcast(1/l).
  P3: out[t, :] = sum_h ctx_h[:, t].T @ Wo_h accumulated in PSUM, streamed
     in two Wo column halves.

The 1/sqrt(HD) score scale is folded into Wq on the host.
"""

import math
import numpy as np
import ml_dtypes
from contextlib import ExitStack

import concourse.bacc as bacc
import concourse.mybir as mybir
import concourse.tile as tile
from concourse.bass_utils import run_bass_kernel_spmd

B, T, D = 4, 2048, 2048
H, HD = 16, 128
P = 128
N_CORES = 8
HEADS_PER_CORE = H // 2          # 8 heads per core (head-group split)
DL = HEADS_PER_CORE * HD         # 1024 local projection dims per core
KC = D // P                      # 16 contraction chunks
TT = T // P                      # 16 token tiles of 128
QC = T // 512                    # 4 query chunks of 512
EXP_BIAS = -10.0                 # exp(s + EXP_BIAS); cancels in normalization

F32 = mybir.dt.float32
BF16 = mybir.dt.bfloat16
BF16_NP = ml_dtypes.bfloat16

LAG = 2                          # P2 head h emitted alongside P1 head h+LAG
SLEAD = 3                        # score-matmul lead over l/c matmuls in P2

_CACHE = {}


def _build(repeat=1):
    nc = bacc.Bacc(None, target_bir_lowering=False)

    xT = nc.dram_tensor("xT", [D, T], BF16, kind="ExternalInput")
    wqh = nc.dram_tensor("wqh", [HEADS_PER_CORE, P, KC, P], BF16, kind="ExternalInput")
    wkh = nc.dram_tensor("wkh", [HEADS_PER_CORE, P, KC, P], BF16, kind="ExternalInput")
    wvh = nc.dram_tensor("wvh", [HEADS_PER_CORE // 2, P, KC, 256], BF16, kind="ExternalInput")
    woT = nc.dram_tensor("woT", [DL, D], BF16, kind="ExternalInput")
    out = nc.dram_tensor("out", [T, D], BF16, kind="ExternalOutput")

    with tile.TileContext(nc) as tc:
        with ExitStack() as octx:
            xp = octx.enter_context(tc.tile_pool(name="xp", bufs=1))
            qkp = octx.enter_context(tc.tile_pool(name="qkp", bufs=3))
            vp = octx.enter_context(tc.tile_pool(name="vp", bufs=3))
            ctxp = octx.enter_context(tc.tile_pool(name="ctxp", bufs=8))
            wqkp = octx.enter_context(tc.tile_pool(name="wqkp", bufs=2))
            wvp = octx.enter_context(tc.tile_pool(name="wvp", bufs=2))
            wop = octx.enter_context(tc.tile_pool(name="wop", bufs=10))
            pp = octx.enter_context(tc.tile_pool(name="pp", bufs=6))
            otp = octx.enter_context(tc.tile_pool(name="otp", bufs=2))
            nrm = octx.enter_context(tc.tile_pool(name="nrm", bufs=2))
            msc = octx.enter_context(tc.tile_pool(name="msc", bufs=1))
            ps_a = octx.enter_context(tc.tile_pool(name="ps_a", bufs=2, space="PSUM"))
            ps_s = octx.enter_context(tc.tile_pool(name="ps_s", bufs=3, space="PSUM"))
            ps_c = octx.enter_context(tc.tile_pool(name="ps_c", bufs=2, space="PSUM"))
            ps_l = octx.enter_context(tc.tile_pool(name="ps_l", bufs=1, space="PSUM"))

            ones = msc.tile([P, 1], BF16, tag="ones")
            nc.vector.memset(ones[:], 1.0)
            ebias = msc.tile([P, 1], F32, tag="ebias")
            nc.vector.memset(ebias[:], EXP_BIAS)

            xT_r = xT.rearrange("(kc p) t -> p kc t", p=P)

            for _rep in range(repeat):
                x_t = xp.tile([P, KC, T], BF16, tag="x")
                for kc in range(KC):
                    nc.sync.dma_start(x_t[:, kc, :], xT_r[:, kc, :])

                q_tiles, k_tiles, v_tiles, ctx_tiles = {}, {}, {}, {}

                def emit_p1_head(h):
                    for wsrc, store in ((wqh, q_tiles), (wkh, k_tiles)):
                        wm = wqkp.tile([P, KC, P], BF16, tag="wqk")
                        nc.sync.dma_start(wm[:], wsrc[h])
                        dst = qkp.tile([P, T], BF16,
                                       tag="q" if store is q_tiles else "k")
                        store[h] = dst
                        for t4 in range(QC):
                            ps = ps_a.tile([P, 512], F32, tag="psa")
                            for kc in range(KC):
                                nc.tensor.matmul(
                                    ps[:],
                                    wm[:, kc, :],
                                    x_t[:, kc, t4 * 512:(t4 + 1) * 512],
                                    start=(kc == 0),
                                    stop=(kc == KC - 1),
                                )
                            nc.vector.tensor_copy(
                                dst[:, t4 * 512:(t4 + 1) * 512], ps[:])
                    if h % 2 == 1:
                        j = h // 2
                        wv = wvp.tile([P, KC, 256], BF16, tag="wv")
                        nc.sync.dma_start(wv[:], wvh[j])
                        vt = vp.tile([P, TT, 256], BF16, tag="v")
                        v_tiles[j] = vt
                        for tt in range(TT):
                            ps = ps_a.tile([P, 256], F32, tag="psa")
                            for kc in range(KC):
                                nc.tensor.matmul(
                                    ps[:],
                                    x_t[:, kc, tt * P:(tt + 1) * P],
                                    wv[:, kc, :],
                                    start=(kc == 0),
                                    stop=(kc == KC - 1),
                                )
                            nc.vector.tensor_copy(vt[:, tt, :], ps[:])

                def emit_p2_head(h):
                    qh = q_tiles.pop(h)
                    kh = k_tiles.pop(h)
                    vt = v_tiles[h // 2]
                    hs = (h % 2) * P
                    ctx_h = ctxp.tile([P, T], BF16, tag="ctx")
                    ctx_tiles[h] = ctx_h

                    for qc in range(QC):
                        nkt = 4 * qc + 4
                        l_ps = ps_l.tile([1, 512], F32, tag="l")
                        c_ps = ps_c.tile([P, 512], F32, tag="c")

                        pTs = {}

                        def emit_s(ki):
                            s_ps = ps_s.tile([P, 512], F32, tag="s")
                            nc.tensor.matmul(
                                s_ps[:],
                                kh[:, ki * P:(ki + 1) * P],
                                qh[:, qc * 512:(qc + 1) * 512],
                                start=True,
                                stop=True,
                            )
                            pT = pp.tile([P, 512], BF16, tag="pT")
                            nc.scalar.activation(
                                pT[:], s_ps[:],
                                mybir.ActivationFunctionType.Exp,
                                bias=ebias[:], scale=1.0,
                            )
                            j = ki - 4 * qc
                            if j >= 0:
                                # keep iff q_rel - k_rel - 128*j >= 0
                                nc.gpsimd.affine_select(
                                    out=pT[:], in_=pT[:],
                                    compare_op=mybir.AluOpType.is_ge,
                                    fill=0.0, base=-P * j,
                                    channel_multiplier=-1,
                                    pattern=[[1, 512]],
                                )
                            pTs[ki] = pT

                        for ki in range(min(SLEAD, nkt)):
                            emit_s(ki)
                        for ki in range(nkt):
                            if ki + SLEAD < nkt:
                                emit_s(ki + SLEAD)
                            pT = pTs.pop(ki)
                            nc.tensor.matmul(
                                l_ps[:], ones[:], pT[:],
                                start=(ki == 0), stop=(ki == nkt - 1),
                            )
                            nc.tensor.matmul(
                                c_ps[:], vt[:, ki, hs:hs + P], pT[:],
                                start=(ki == 0), stop=(ki == nkt - 1),
                            )
                        rl = nrm.tile([1, 512], F32, tag="rl")
                        nc.vector.reciprocal(rl[:], l_ps[:])
                        rb = nrm.tile([P, 512], F32, tag="rb")
                        nc.gpsimd.partition_broadcast(rb[:], rl[:])
                        nc.vector.tensor_mul(
                            ctx_h[:, qc * 512:(qc + 1) * 512], c_ps[:], rb[:])
                    if h % 2 == 1:
                        v_tiles.pop(h // 2)

                for h in range(HEADS_PER_CORE + LAG):
                    if h < HEADS_PER_CORE:
                        emit_p1_head(h)
                    if h >= LAG:
                        emit_p2_head(h - LAG)

                # ---------------- P3: output projection ----------------
                for ocH in range(2):
                    wo_t = []
                    for h in range(HEADS_PER_CORE):
                        wt = wop.tile([P, 1024], BF16, tag="wo")
                        nc.sync.dma_start(
                            wt[:], woT[h * P:(h + 1) * P,
                                       ocH * 1024:(ocH + 1) * 1024])
                        wo_t.append(wt)
                    for tt in range(TT):
                        ot = otp.tile([P, 1024], BF16, tag="ot")
                        for oc2 in range(2):
                            ps = ps_a.tile([P, 512], F32, tag="psa")
                            for h in range(HEADS_PER_CORE):
                                nc.tensor.matmul(
                                    ps[:],
                                    ctx_tiles[h][:, tt * P:(tt + 1) * P],
                                    wo_t[h][:, oc2 * 512:(oc2 + 1) * 512],
                                    start=(h == 0),
                                    stop=(h == HEADS_PER_CORE - 1),
                                )
                            nc.vector.tensor_copy(
                                ot[:, oc2 * 512:(oc2 + 1) * 512], ps[:])
                        nc.sync.dma_start(
                            out[tt * P:(tt + 1) * P,
                                ocH * 1024:(ocH + 1) * 1024], ot[:])

    nc.compile()
    return nc


def _get_nc(repeat=1):
    if repeat not in _CACHE:
        _CACHE[repeat] = _build(repeat)
    return _CACHE[repeat]


def make_in_maps(inputs):
    x = np.asarray(inputs["x"], dtype=np.float32)
    Wq = np.asarray(inputs["Wq"], dtype=np.float32)
    Wk = np.asarray(inputs["Wk"], dtype=np.float32)
    Wv = np.asarray(inputs["Wv"], dtype=np.float32)
    Wo = np.asarray(inputs["Wo"], dtype=np.float32)

    scale = 1.0 / math.sqrt(HD)

    def heads4(A, grp):
        # A: [DL, D] -> [n_grp, P_partition, KC, grp] with
        # out[j, p, kc, m] = A[j*grp + m, kc*128 + p]
        n = DL // grp
        return np.ascontiguousarray(
            A.reshape(n, grp, KC, P).transpose(0, 3, 2, 1).astype(BF16_NP))

    in_maps = []
    for c in range(N_CORES):
        b, g = divmod(c, 2)
        hs = slice(g * DL, (g + 1) * DL)
        in_maps.append({
            "xT": np.ascontiguousarray(x[b].T.astype(BF16_NP)),
            "wqh": heads4(Wq[hs, :] * scale, P),
            "wkh": heads4(Wk[hs, :], P),
            "wvh": heads4(Wv[hs, :], 256),
            "woT": np.ascontiguousarray(Wo[:, hs].T.astype(BF16_NP)),
        })
    return in_maps


def run(inputs, trace=False, repeat=1):
    in_maps = make_in_maps(inputs)
    b_out = np.asarray(inputs["b_out"], dtype=np.float32)

    nc = _get_nc(repeat)
    res = run_bass_kernel_spmd(nc, in_maps, core_ids=list(range(N_CORES)),
                               trace=trace)
    outp = np.empty((B, T, D), dtype=np.float32)
    for b in range(B):
        outp[b] = (res.results[2 * b]["out"].astype(np.float32)
                   + res.results[2 * b + 1]["out"].astype(np.float32))
    outp += b_out[None, None, :]
    return outp, res


def kernel(**inputs) -> np.ndarray:
    outp, _ = run(inputs, trace=False)
    return outp
